# revision 23
# baseline (speedup 1.0000x reference)
"""GBST layer (pooling) Trainium2 Bass/Tile kernel.

Math (per sample, x [512, 8192]):
  y = conv1d(x, W[512,512,5], b, VALID)                    # [512, 8188]
  r[l] = score . y[:, l]                                   # conv'd scores
  For w in {1,2,3}: cand_w = block-mean(y, w); s_w = block-mean(r, w)
  att = softmax over the 3 per-position scores; out[l] = sum_w att_w[l] * cand_w(bw(l))
  out_ds = pairwise mean of out                            # [512, 4096]

Strategy: 1 sample per NeuronCore (8 cores, batch-parallel).
  - conv as 5 shifted bf16 matmuls per (oc, ic) chunk pair on PE (1280 MMs);
    y kept resident in ONE SBUF tile [128, 4*8208] bf16 (chunk-major cols)
    so post-conv elementwise ops merge all 4 channel chunks per instruction
  - r via PE (score^T @ y) in fp8e4 DoubleRow mode: each conv tile's PSUM
    is read twice by ACT (bf16 y + fp8 y8 copies) and the score row needs
    2 DR matmuls per tile instead of 4 bf16 ones.  The DR weight pair-dim
    step must be even and 16B-aligned (s3_lw_dual_fp8_restrictions), so
    the four fp8 score chunks live 16B apart in sc_sb.  fp8 only perturbs
    the softmax scores, not the candidate values (end-to-end rel err
    0.0070 vs the 2e-2 gate).  r is stored to DRAM and re-gathered in the
    L6 layout for the coefficient (softmax) math
  - coefficient rows are emitted INTERLEAVED per output position:
      ABi[2p]=A[p], ABi[2p+1]=B[p];  DEi[2p]=D[p], DEi[2p+1]=E[p]  (bf16)
    because with j = 2p+s the softmax/downsample combine collapses to
      m[j]   = ABi[j]*y[j] + DEi[j]*S3[floor(j/3)]
      out[p] = m[2p] + m[2p+1]
    where S3[g] = y[3g]+y[3g+1]+y[3g+2].  floor(j/3) is the repeat-3 AP
    ([..,[1,G],[0,3]]), so the whole combine for a 768-position group is
    6 wide instructions over [128, 4*1536] bf16: the strided ops (S3
    build, final pair-sum) ride the otherwise-idle GPSIMD, the packed
    muls/adds stay on the DVE where their stride-1 bf16 last dims hit the
    2x path.  DVE instruction count drops ~4.5x vs the per-term strided
    formulation (867 -> 192 per iteration)
  - final out tile is bf16; the SWDGE store casts to f32 on the way out
    (measured end-to-end rel err 0.0039 vs the 2e-2 gate)
  - dependent DMAs (r stores, gathers, scatters, broadcasts, out stores)
    ride the SWDGE/Pool ring so their semaphore waits cannot head-of-line
    block the PE-critical x/weight loads on the SP HWDGE ring
  - schedule: conv tiles 0..15 with emit_r lagging one tile; coefficient
    groups {0,1},{2,3},{4,5},{6,7},{8},{9},{10} (768-position blocks) are
    gathered/mathed/broadcast/combined as soon as their r rows exist;
    emit_r(14) is hoisted ahead of conv(15) so group 9 starts early and
    only group 10 plus the last combines trail the PE stream

This walrus build caps semaphore waits per instruction very low, so
_fix_wait_overflow() hoists excess waits onto injected same-engine NOPs
placed immediately before the overflowing instruction (safe: no intervening
same-engine instructions, so producers can't depend on anything between).
"""

import numpy as np
import ml_dtypes

import concourse.bass as bass
import concourse.mybir as mybir
from concourse.tile import TileContext

BF16 = mybir.dt.bfloat16
F8 = mybir.dt.float8e4
F32 = mybir.dt.float32
AF = mybir.ActivationFunctionType
ALU = mybir.AluOpType

N_CORES = 8
E, L, KS = 512, 8192, 5
LC = L - KS + 1          # 8188 valid conv outputs
LPAD = 8208              # y cols incl. zero tail (S3 reads up to col 8204)
NCB = 11                 # 768-r-position blocks: 384 out cols each
RPAD = 768 * NCB         # padded r length
PCO = 768 * NCB          # interleaved coef row length (2 per out col)
NT = 16                  # conv tiles of 512 positions
OUTL = L // 2            # 4096

# coefficient/combine groups: (first block c0, n blocks). 768 r rows per
# block -> 384 out cols per block. Group g is ready once its last r row
# (768*(c0+nb)-1) has been stored, i.e. after conv tile (768*(c0+nb)-1)//512.
GROUPS = [(0, 2), (2, 2), (4, 2), (6, 2), (8, 1), (9, 1), (10, 1)]

_BUILT = None
_PRUNE_LEVEL = 2         # probe hook: 0=conv+r, 1=+phase2, 2=full kernel


def _sap(tile_ap, col_off, dims):
    """Strided SBUF AP on a pool tile: partition dim + custom free dims."""
    pitch, nparts = tile_ap.ap[0]
    return bass.AP(tile_ap.tensor, tile_ap.offset + col_off, [[pitch, nparts]] + dims)


def _fix_wait_overflow(nc):
    """Split >limit semaphore waits onto injected same-engine NOPs."""
    cnt = 0
    for f in nc.m.functions:
        for b in f.blocks:
            newlist = []
            for inst in b.instructions:
                si = inst.sync_info
                if si is not None and si.on_wait:
                    lim = 1
                    waits = list(si.on_wait)
                    while len(waits) > lim:
                        w = waits.pop(0)
                        nop = mybir.InstNoOp(name=f"wfx-{cnt}")
                        cnt += 1
                        nop.engine = inst.engine
                        nop.sync_info = mybir.SyncInfo(on_wait=[w], on_update=[])
                        newlist.append(nop)
                    if cnt and len(waits) != len(si.on_wait):
                        inst.sync_info = mybir.SyncInfo(
                            on_wait=waits, on_update=list(si.on_update)
                        )
                newlist.append(inst)
            b.instructions[:] = newlist
    return cnt


def _build_bass(fix_waits=True, reps=1, rep_barrier=False):
    nc = bass.Bass("TRN2", target_bir_lowering=False, num_devices=N_CORES)

    xb = nc.dram_tensor("xb", [E, L], BF16, kind="ExternalInput")
    # weights laid out so oc is the OUTER index: chunk (oc,k,ic) at
    # col ((oc*KS + k)*4 + ic)*128  -> per-oc slices are contiguous
    wsb = nc.dram_tensor("wsb", [128, KS * 4 * 4 * 128], BF16, kind="ExternalInput")
    # fp8 score chunks spaced 16B apart: the DoubleRow weight pair-dim
    # step must be even and 16B-aligned (s3_lw_dual_fp8_restrictions)
    scs = nc.dram_tensor("scs", [128, 64], F8, kind="ExternalInput")
    bis = nc.dram_tensor("bis", [128, 4], F32, kind="ExternalInput")
    out_d = nc.dram_tensor("out", [E, OUTL], F32, kind="ExternalOutput")
    coefab_d = nc.dram_tensor("coefab", [PCO], BF16)
    coefde_d = nc.dram_tensor("coefde", [PCO], BF16)
    r_d = nc.dram_tensor("r_scratch", [RPAD], F32)

    with TileContext(nc) as tc:
        with (
            tc.tile_pool(name="const", bufs=1) as kpool,
            tc.tile_pool(name="ybuf", bufs=1) as ypool,
            tc.tile_pool(name="xin", bufs=3) as xpool,
            tc.tile_pool(name="ps", bufs=4, space="PSUM") as pspool,
            tc.tile_pool(name="psr", bufs=2, space="PSUM") as prpool,
            tc.tile_pool(name="sc", bufs=2) as spool,
            tc.tile_pool(name="cf", bufs=2) as cpool,
            tc.tile_pool(name="s3", bufs=2) as s3pool,
            tc.tile_pool(name="mm", bufs=2) as mpool,
            tc.tile_pool(name="ot", bufs=2) as opool,
            tc.tile_pool(name="y8", bufs=2) as y8pool,
        ):
            # DMA order tuned for time-to-first-matmul: x tile 0, then the
            # five (oc0,k) weight pieces the first PSUM group consumes (in
            # k order, matching the k-major matmul loop), then the rest.
            xts = {}

            def load_x(t):
                if t >= NT:
                    return
                n0 = 512 * t
                xw = min(516, L - n0)
                xt = xpool.tile([128, 4 * 516], BF16, tag="xt")
                nc.sync.dma_start(
                    out=_sap(xt, 0, [[516, 4], [1, xw]]),
                    in_=bass.AP(xb, n0, [[L, 128], [128 * L, 4], [1, xw]]),
                )
                xts[t] = xt

            load_x(0)

            # 20 (oc,k) pieces so the first matmuls only wait on 131 KB
            w_sb = kpool.tile([128, KS * 4 * 4 * 128], BF16, tag="w")
            for k in range(KS):
                c0 = k * 512
                nc.sync.dma_start(
                    out=w_sb[:, c0 : c0 + 512], in_=wsb[:, c0 : c0 + 512]
                )

            sc_sb = kpool.tile([128, 64], F8, tag="sc")
            nc.sync.dma_start(out=sc_sb[:], in_=scs[:])
            bi_sb = kpool.tile([128, 4], F32, tag="bi")
            nc.sync.dma_start(out=bi_sb[:], in_=bis[:])
            load_x(1)

            for oc in range(1, 4):
                for k in range(KS):
                    c0 = (oc * KS + k) * 512
                    nc.sync.dma_start(
                        out=w_sb[:, c0 : c0 + 512], in_=wsb[:, c0 : c0 + 512]
                    )

            # ONE y tile, chunk-major: chunk cc at cols [cc*LPAD, (cc+1)*LPAD)
            y_all = ypool.tile([128, 4 * LPAD], BF16, name="y_all", tag="y_all")
            for cc in range(4):
                nc.gpsimd.memset(y_all[:, cc * LPAD + LC : (cc + 1) * LPAD], 0.0)
            zr = kpool.tile([1, RPAD - LC], F32, tag="zr")
            nc.gpsimd.memset(zr[:], 0.0)
            nc.gpsimd.dma_start(out=bass.AP(r_d, LC, [[1, RPAD - LC]]), in_=zr[:1, :])

            for _rep in range(reps):
                _emit_body(nc, tc, w_sb, sc_sb, bi_sb, y_all, xb, out_d,
                           coefab_d, coefde_d, r_d,
                           xpool, pspool, prpool, spool, cpool, s3pool,
                           mpool, opool, y8pool, xts, load_x)
                if rep_barrier and _rep != reps - 1:
                    # serialize reps: turns the rep slope into a single-shot
                    # (non-pipelined) time for benchmarking
                    nc.all_engine_barrier()

    if fix_waits:
        _fix_wait_overflow(nc)
    return nc


def _emit_body(nc, tc, w_sb, sc_sb, bi_sb, y_all, xb, out_d, coefab_d, coefde_d,
               r_d, xpool, pspool, prpool, spool, cpool, s3pool, mpool, opool,
               y8pool, xts, load_x):
    # rep boundary: tiles 0/1 are prefetched either at build start (first
    # rep) or by the previous rep's wrap-around prefetch below.
    for t0 in (0, 1):
        if t0 not in xts:
            load_x(t0)

    y8ts = {}

    def conv_tile(t):
        n0 = 512 * t
        n = min(512, LC - n0)
        xt = xts.pop(t)
        if (t + 2) % NT not in xts:
            load_x((t + 2) % NT)  # wrap-around also prefetches the next rep
        y8t = y8pool.tile([128, 2048], F8, tag="y8t")
        y8ts[t] = y8t
        for oc in range(4):
            py = pspool.tile([128, 512], F32, tag="py")
            first = True
            for k in range(KS):
                for ic in range(4):
                    nc.tensor.matmul(
                        py[:, :n],
                        lhsT=w_sb[
                            :,
                            ((oc * KS + k) * 4 + ic) * 128 : ((oc * KS + k) * 4 + ic + 1) * 128,
                        ],
                        rhs=xt[:, ic * 516 + k : ic * 516 + k + n],
                        start=first,
                        stop=(k == KS - 1 and ic == 3),
                    )
                    first = False
            nc.scalar.activation(
                y_all[:, oc * LPAD + n0 : oc * LPAD + n0 + n], py[:, :n],
                AF.Identity, bias=bi_sb[:, oc : oc + 1], scale=1.0,
            )
            # second PSUM read: biased y cast to fp8 so the score row can
            # use DoubleRow matmuls (2 instructions instead of 4); fp8 only
            # perturbs the softmax scores, not the candidate values
            nc.scalar.activation(
                y8t[:, oc * 512 : oc * 512 + n], py[:, :n],
                AF.Identity, bias=bi_sb[:, oc : oc + 1], scale=1.0,
            )

    def emit_r(t):
        n0 = 512 * t
        n = min(512, LC - n0)
        y8t = y8ts.pop(t)
        pr = prpool.tile([1, 512], F32, tag="pr")
        for p in range(2):
            nc.tensor.matmul(
                pr[:, :n],
                lhsT=_sap(sc_sb, 32 * p, [[16, 2], [1, 1]]),
                rhs=_sap(y8t, 1024 * p, [[512, 2], [1, n]]),
                start=(p == 0),
                stop=(p == 1),
                perf_mode=mybir.MatmulPerfMode.DoubleRow,
            )
        rsb = xpool.tile([1, 512], F32, tag="rsb")
        nc.scalar.activation(rsb[:1, :n], pr[:1, :n], AF.Copy)
        # SWDGE ring: this store waits on the ACT copy; on the SP ring that
        # wait would head-of-line-block the PE-critical xt/weight loads.
        nc.gpsimd.dma_start(out=bass.AP(r_d, n0, [[1, n]]), in_=rsb[:1, :n])

    r6s = {}

    def phase2_load(gi):
        c0, nb = GROUPS[gi]
        # r6[j, 6cb+u] = r[768(c0+cb) + 6j + u]; one DMA, 24B runs
        r6 = spool.tile([128, 12], F32, tag="r6")
        nc.gpsimd.dma_start(
            out=_sap(r6, 0, [[6, nb], [1, 6]]),
            in_=bass.AP(r_d, 768 * c0, [[6, 128], [768, nb], [1, 6]]),
        )
        r6s[gi] = r6

    def phase2_math(gi):
        c0, nb = GROUPS[gi]
        r6 = r6s.pop(gi)
        w6, w3, w2 = 6 * nb, 3 * nb, 2 * nb
        e1 = spool.tile([128, 12], F32, tag="e1")
        nc.scalar.activation(e1[:, :w6], r6[:, :w6], AF.Exp)
        # s2h[j,3cb+v] = r6[,6cb+2v] + r6[,6cb+2v+1];  e2 = exp(s2h/2)
        s2h = spool.tile([128, 6], F32, tag="s2h")
        nc.vector.tensor_add(
            out=s2h[:, :w3],
            in0=_sap(r6, 0, [[6, nb], [2, 3]]),
            in1=_sap(r6, 1, [[6, nb], [2, 3]]),
        )
        e2 = spool.tile([128, 6], F32, tag="e2")
        nc.scalar.activation(e2[:, :w3], s2h[:, :w3], AF.Exp, scale=0.5)
        # s3h[j,2cb+w] = sum of r6[,6cb+3w+{0,1,2}];  e3 = exp(s3h/3)
        s3h = spool.tile([128, 4], F32, tag="s3h")
        nc.vector.tensor_add(
            out=s3h[:, :w2],
            in0=_sap(r6, 0, [[6, nb], [3, 2]]),
            in1=_sap(r6, 1, [[6, nb], [3, 2]]),
        )
        nc.vector.tensor_add(
            out=s3h[:, :w2], in0=s3h[:, :w2], in1=_sap(r6, 2, [[6, nb], [3, 2]])
        )
        e3 = spool.tile([128, 4], F32, tag="e3")
        nc.scalar.activation(e3[:, :w2], s3h[:, :w2], AF.Exp, scale=1.0 / 3.0)

        # den = e1 + expand2(e2) + expand3(e3), then rec = 1/den
        den = spool.tile([128, 12], F32, tag="den")
        for v in range(3):
            nc.vector.tensor_add(
                out=_sap(den, 2 * v, [[6, nb], [1, 2]]),
                in0=_sap(e1, 2 * v, [[6, nb], [1, 2]]),
                in1=_sap(e2, v, [[3, nb], [0, 2]]),
            )
        for w in range(2):
            nc.vector.tensor_add(
                out=_sap(den, 3 * w, [[6, nb], [1, 3]]),
                in0=_sap(den, 3 * w, [[6, nb], [1, 3]]),
                in1=_sap(e3, w, [[2, nb], [0, 3]]),
            )
        rec = spool.tile([128, 12], F32, tag="rec")
        nc.vector.reciprocal(rec[:, :w6], den[:, :w6])

        # t1 = e1*rec ; recsum[3cb+v] = rec[6cb+2v]+rec[6cb+2v+1]
        t1 = spool.tile([128, 12], F32, tag="t1")
        nc.vector.tensor_mul(out=t1[:, :w6], in0=e1[:, :w6], in1=rec[:, :w6])
        recsum = spool.tile([128, 6], F32, tag="recsum")
        nc.vector.tensor_add(
            out=recsum[:, :w3],
            in0=_sap(rec, 0, [[6, nb], [2, 3]]),
            in1=_sap(rec, 1, [[6, nb], [2, 3]]),
        )
        # e2r = 0.25 * e2 * recsum
        e2r = spool.tile([128, 6], F32, tag="e2r")
        nc.vector.scalar_tensor_tensor(
            out=e2r[:, :w3], in0=e2[:, :w3], scalar=0.25, in1=recsum[:, :w3],
            op0=ALU.mult, op1=ALU.mult,
        )
        # interleaved coef rows (bf16): ABi[,6cb+2v+s] = (A,B)[384(c0+cb)+3j+v]
        ABi = spool.tile([128, 12], BF16, tag="ABi")
        nc.vector.scalar_tensor_tensor(
            out=_sap(ABi, 0, [[6, nb], [2, 3]]),
            in0=_sap(t1, 0, [[6, nb], [2, 3]]),
            scalar=0.5, in1=e2r[:, :w3], op0=ALU.mult, op1=ALU.add,
        )
        nc.vector.scalar_tensor_tensor(
            out=_sap(ABi, 1, [[6, nb], [2, 3]]),
            in0=_sap(t1, 1, [[6, nb], [2, 3]]),
            scalar=0.5, in1=e2r[:, :w3], op0=ALU.mult, op1=ALU.add,
        )
        # D[3cb+v] = (1/6) e3[2cb + (0,0,1)v] * rec[6cb+2v]
        # E[3cb+v] = (1/6) e3[2cb + (0,1,1)v] * rec[6cb+2v+1]
        DEi = spool.tile([128, 12], BF16, tag="DEi")
        for v, (w0, w1) in enumerate([(0, 0), (0, 1), (1, 1)]):
            nc.vector.scalar_tensor_tensor(
                out=_sap(DEi, 2 * v, [[6, nb]]),
                in0=_sap(e3, w0, [[2, nb]]),
                scalar=1.0 / 6.0,
                in1=_sap(rec, 2 * v, [[6, nb]]),
                op0=ALU.mult, op1=ALU.mult,
            )
            nc.vector.scalar_tensor_tensor(
                out=_sap(DEi, 2 * v + 1, [[6, nb]]),
                in0=_sap(e3, w1, [[2, nb]]),
                scalar=1.0 / 6.0,
                in1=_sap(rec, 2 * v + 1, [[6, nb]]),
                op0=ALU.mult, op1=ALU.mult,
            )
        # scatter interleaved rows to DRAM in j-natural order (12B runs)
        nc.gpsimd.dma_start(
            out=bass.AP(coefab_d, 768 * c0, [[6, 128], [768, nb], [1, 6]]),
            in_=ABi[:, :w6],
        )
        nc.gpsimd.dma_start(
            out=bass.AP(coefde_d, 768 * c0, [[6, 128], [768, nb], [1, 6]]),
            in_=DEi[:, :w6],
        )

    cbts = {}

    def combine_load(gi):
        c0, nb = GROUPS[gi]
        W = min(384 * nb, OUTL - 384 * c0)
        J = 2 * W
        ABb = cpool.tile([128, 1536], BF16, tag="ABb")
        nc.gpsimd.dma_start(
            out=ABb[:, :J], in_=bass.AP(coefab_d, 768 * c0, [[0, 128], [1, J]])
        )
        DEb = cpool.tile([128, 1536], BF16, tag="DEb")
        nc.gpsimd.dma_start(
            out=DEb[:, :J], in_=bass.AP(coefde_d, 768 * c0, [[0, 128], [1, J]])
        )
        cbts[gi] = (ABb, DEb)

    def combine_math(gi):
        c0, nb = GROUPS[gi]
        p0 = 384 * c0
        W = min(384 * nb, OUTL - p0)
        J = 2 * W                 # y cols consumed (j = 2p+s)
        j0 = 768 * c0
        G = (J + 2) // 3          # S3 cols needed
        Gr = J // 3               # full repeat-3 triples
        ABb, DEb = cbts.pop(gi)
        # S3[g] = y[3g] + y[3g+1] + y[3g+2], g in [256*c0, 256*c0+G)
        # (strided reads gain nothing from the DVE 2x path -> GPSIMD, which
        # is otherwise idle between its DMA triggers; materializing the
        # repeat-3 on GPSIMD instead was tried and measurably regresses)
        s3t = s3pool.tile([128, 2048], BF16, tag="s3t")
        nc.gpsimd.tensor_add(
            out=_sap(s3t, 0, [[512, 4], [1, G]]),
            in0=_sap(y_all, j0, [[LPAD, 4], [3, G]]),
            in1=_sap(y_all, j0 + 1, [[LPAD, 4], [3, G]]),
        )
        nc.gpsimd.tensor_add(
            out=_sap(s3t, 0, [[512, 4], [1, G]]),
            in0=_sap(s3t, 0, [[512, 4], [1, G]]),
            in1=_sap(y_all, j0 + 2, [[LPAD, 4], [3, G]]),
        )
        # m = ABi*y + DEi*repeat3(S3)  (bf16, packed last dims)
        mab = mpool.tile([128, 6144], BF16, tag="mab")
        nc.vector.tensor_mul(
            out=_sap(mab, 0, [[1536, 4], [1, J]]),
            in0=_sap(y_all, j0, [[LPAD, 4], [1, J]]),
            in1=_sap(ABb, 0, [[0, 4], [1, J]]),
        )
        mde = mpool.tile([128, 6144], BF16, tag="mde")
        nc.vector.tensor_mul(
            out=_sap(mde, 0, [[1536, 4], [1, 3 * Gr]]),
            in0=_sap(s3t, 0, [[512, 4], [1, Gr], [0, 3]]),
            in1=_sap(DEb, 0, [[0, 4], [1, 3 * Gr]]),
        )
        if 3 * Gr < J:  # tail group: J % 3 != 0
            rem = J - 3 * Gr
            nc.vector.tensor_mul(
                out=_sap(mde, 3 * Gr, [[1536, 4], [1, rem]]),
                in0=_sap(s3t, Gr, [[512, 4], [0, rem]]),
                in1=_sap(DEb, 3 * Gr, [[0, 4], [1, rem]]),
            )
        nc.vector.tensor_add(
            out=_sap(mab, 0, [[1536, 4], [1, J]]),
            in0=_sap(mab, 0, [[1536, 4], [1, J]]),
            in1=_sap(mde, 0, [[1536, 4], [1, J]]),
        )
        # out[p] = m[2p] + m[2p+1]; SWDGE store casts bf16 -> f32. The
        # pair-sum is stride-2 (no DVE 2x) and feeds a same-queue store ->
        # GPSIMD keeps the whole epilogue off the DVE critical path.
        ot = opool.tile([128, 3072], BF16, tag="ot")
        nc.gpsimd.tensor_add(
            out=_sap(ot, 0, [[768, 4], [1, W]]),
            in0=_sap(mab, 0, [[1536, 4], [2, W]]),
            in1=_sap(mab, 1, [[1536, 4], [2, W]]),
        )
        nc.gpsimd.dma_start(
            out=bass.AP(out_d, p0, [[OUTL, 128], [128 * OUTL, 4], [1, W]]),
            in_=_sap(ot, 0, [[768, 4], [1, W]]),
        )

    # ---- interleaved emission schedule ----
    # group g ready (r rows stored) after conv tile (768*(c0+nb)-1)//512;
    # with the emit_r one-tile lag the gather goes one tile later, math the
    # tile after, broadcast next, combine one more tile later (each stage
    # >=1 conv tile after its producer so nothing stalls mid-stream).
    P2L_AT = {3: [0], 6: [1], 9: [2], 12: [3], 14: [4], 15: [5]}
    P2M_AT = {4: [0], 7: [1], 10: [2], 13: [3], 15: [4]}
    CBL_AT = {5: [0], 8: [1], 11: [2], 14: [3], 15: [4]}
    CBM_AT = {6: [0], 9: [1], 12: [2], 15: [3]}
    lvl = _PRUNE_LEVEL
    for t in range(NT):
        if t == NT - 1:
            # pull r(14) ahead of conv(15)'s 80 matmuls: group 5 (block c9)
            # then hides under the last conv tile.
            emit_r(t - 1)
        conv_tile(t)
        if 1 <= t < NT - 1:
            emit_r(t - 1)
        if lvl >= 1:
            for g in P2L_AT.get(t, []):
                phase2_load(g)
            for g in P2M_AT.get(t, []):
                phase2_math(g)
        if lvl >= 2:
            for g in CBL_AT.get(t, []):
                combine_load(g)
            for g in CBM_AT.get(t, []):
                combine_math(g)
    # tail: only group 6 (block c10) depends on conv(15)'s r rows
    emit_r(NT - 1)
    if lvl >= 1:
        phase2_load(6)
    if lvl >= 2:
        combine_math(4)
    if lvl >= 1:
        phase2_math(5)
    if lvl >= 2:
        combine_load(5)
        combine_math(5)
    if lvl >= 1:
        phase2_math(6)
    if lvl >= 2:
        combine_load(6)
        combine_math(6)


def _prep_inputs(x, conv_w, conv_b, score_w):
    """Per-core input maps. Core b processes sample b."""
    bf = ml_dtypes.bfloat16
    wT = np.ascontiguousarray(conv_w.transpose(1, 0, 2))  # [in, out, k]
    wsb = np.empty((128, KS * 4 * 4 * 128), dtype=bf)
    for oc in range(4):
        for k in range(KS):
            for ic in range(4):
                off = ((oc * KS + k) * 4 + ic) * 128
                wsb[:, off : off + 128] = wT[
                    128 * ic : 128 * (ic + 1), 128 * oc : 128 * (oc + 1), k
                ].astype(bf)
    scs = np.zeros((128, 64), dtype=ml_dtypes.float8_e4m3)
    for cc in range(4):
        scs[:, 16 * cc] = score_w.reshape(4, 128)[cc].astype(
            ml_dtypes.float8_e4m3
        )
    bis = np.ascontiguousarray(conv_b.reshape(4, 128).T.astype(np.float32))
    maps = []
    for b in range(N_CORES):
        maps.append({"xb": x[b].astype(bf), "wsb": wsb, "scs": scs, "bis": bis})
    return maps


def kernel(x, conv_w, conv_b, score_w):
    global _BUILT
    from concourse.bass_utils import run_bass_kernel_spmd

    if _BUILT is None:
        _BUILT = _build_bass()
    nc = _BUILT
    x = np.asarray(x, dtype=np.float32)
    maps = _prep_inputs(
        x,
        np.asarray(conv_w, dtype=np.float32),
        np.asarray(conv_b, dtype=np.float32),
        np.asarray(score_w, dtype=np.float32),
    )
    res = run_bass_kernel_spmd(nc, maps, core_ids=list(range(N_CORES)))
    out = np.stack([r["out"] for r in res.results], axis=0)
    return out.astype(np.float32)
